# revision 1
# baseline (speedup 1.0000x reference)
"""Trainium2 Bass kernel for the AG_RoPE-style linear-attention encoder layer.

Distribution (8 NeuronCores): core c -> (batch = c//2, H-half = c%2).
Each core handles one [C, 64, 128] slab of x/out for its batch.
The only cross-core dependency is the linear-attention KV/Ksum reduction over
the *source* tokens of a batch -> tiny [256, 33] AllReduce over core pairs.

Per-core pipeline (channel-major [C, tokens] layouts, fp32 data with
float32r matmuls):
  source: DMA -> 4x4 maxpool -> LN1 -> K/V projections (token-major) ->
          elu(K)+1 -> per-head KV = Kp^T V, Ksum -> pair AllReduce
  x:      processed in two independent row-halves so the MLP can start on the
          first half while the second is still in flight: DMA padded shard
          [C, 72, 136] (clamp-duplicated halo) -> maxpool -> LN1 -> Q ->
          elu+1 -> z = 1/(Qp . Ksum) -> per-head msg = KV^T Qp * z -> merge
          -> pooled message [C, 18, 34] -> H-direction bilinear x4
  out:    per 512-token chunk: W-direction bilinear x4 (pair-batched), MLP
          (relu(cat @ W1) @ W2centered), LN2 via sum-of-squares matmul +
          PE row-broadcast + reciprocal, residual add, DMA out.

The L=S scaling of the reference (V/L ... * L) cancels and is dropped. LN2's
mean-subtraction and norm2_g are folded into W2 on the host; norm2_b is added
in the epilogue.
"""

import sys

for _p in ("/opt/trn_rl_repo",):
    if _p not in sys.path:
        sys.path.insert(0, _p)

import numpy as np

import concourse.bass as bass
import concourse.mybir as mybir
import concourse.tile as tile
from concourse import bacc
from concourse.bass_utils import run_bass_kernel_spmd

F32 = mybir.dt.float32
F32R = mybir.dt.float32r
ALU = mybir.AluOpType
ACTF = mybir.ActivationFunctionType
AX = mybir.AxisListType
BF16 = mybir.dt.bfloat16

C = 256
NH = 8
D = 32
EPS_LN = 1e-5
HL, WF = 64, 128          # rows per core, full width
HP, WP = HL + 8, WF + 8   # padded x shard: 72 x 136
PH, PW = HP // 4, WP // 4  # padded pooled grid 18 x 34
NQ = PH * PW              # 612 q tokens
NS = (HL // 4) * (WF // 4)  # 512 source tokens
NTOK = HL * WF            # 8192 out tokens/core
CHUNK = 512
NCH = NTOK // CHUNK       # 16 chunks (4 rows each)
# bilinear x4 (half-pixel): out[4k+r] = pp[k+s] + w*(pp[k+s+1]-pp[k+s])
UPW = {0: (0, 0.625), 1: (0, 0.875), 2: (1, 0.125), 3: (1, 0.375)}
# x-side halves: (pool-row start, pool rows, token offset, tokens, hu k0)
HALVES = ((0, 10, 0, 340, 0), (10, 8, 340, 272, 8))


def r32(ap):
    return ap.bitcast(F32R)


def f32(ap):
    return ap.bitcast(F32)


def _build_body(nc, tc, io, ctx, use_collective=True):
    xs, src, out = io["xs"], io["src"], io["out"]
    v = nc.vector
    s = nc.scalar
    g = nc.gpsimd
    t = nc.tensor
    sy = nc.sync

    fixed = ctx.enter_context(tc.tile_pool(name="fixed", bufs=1))

    # ---- weights to SBUF (staged f32 DMA -> ACT copy rounds to f32r)
    w1s = fixed.tile([128, 2048], F32R)  # cols = 512*ict + 128*oct + oc
    w2s = fixed.tile([128, 1024], F32R)  # cols = 256*ict + 128*oct + oc
    wqs = fixed.tile([128, 512], F32R)
    wks = fixed.tile([128, 512], F32R)
    wvs = fixed.tile([128, 512], F32R)
    wms = fixed.tile([128, 512], F32R)
    loads = [(w1s, "w1", 4, 512), (w2s, "w2", 4, 256),
             (wqs, "wq", 2, 256), (wks, "wk", 2, 256),
             (wvs, "wv", 2, 256), (wms, "wm", 2, 256)]
    for ws, nm, nt, wd in loads:
        for ict in range(nt):
            g.dma_start(out=ws[:, wd * ict:wd * (ict + 1)],
                        in_=io[nm].ap()[128 * ict:128 * (ict + 1), :])
    # per-channel vectors: cols = g1, b1, b2, 1/g2  ([128, 4] per ctile)
    vec = fixed.tile([128, 8], F32)
    for ct in range(2):
        sy.dma_start(out=vec[:, 4 * ct:4 * (ct + 1)],
                     in_=io["vecs"].ap()[128 * ct:128 * (ct + 1), :])
    g1 = [vec[:, 4 * ct + 0:4 * ct + 1] for ct in range(2)]
    b1 = [vec[:, 4 * ct + 1:4 * ct + 2] for ct in range(2)]
    b2 = [vec[:, 4 * ct + 2:4 * ct + 3] for ct in range(2)]
    ig2 = [vec[:, 4 * ct + 3:4 * ct + 4] for ct in range(2)]

    epsc = fixed.tile([1, 1], F32)
    v.memset(epsc, EPS_LN)
    ones_colF = fixed.tile([128, 1], F32)
    v.memset(ones_colF, 1.0)
    ones_col = fixed.tile([128, 1], F32R)
    s.copy(ones_col, ones_colF)
    ones_rowF = fixed.tile([1, 128], F32)
    v.memset(ones_rowF, 1.0)
    ones_row = fixed.tile([1, 128], F32R)
    s.copy(ones_row, ones_rowF)
    hb = fixed.tile([8, 256], F32R)  # per-head row->32-partition blocks
    g.dma_start(out=hb, in_=io["hbm"].ap())
    ksb = fixed.tile([128, 16], F32R)  # block-diag Ksum cols: 8*ct + head
    zf16 = fixed.tile([128, 16], F32)
    v.memset(zf16, 0.0)
    s.copy(ksb, zf16)

    dram = ctx.enter_context(tc.tile_pool(name="dram", bufs=1, space="DRAM"))
    cc_in = dram.tile([C, 33], F32)
    cc_out = dram.tile([C, 33], F32)

    bridge = ctx.enter_context(tc.tile_pool(name="bridge", bufs=1))
    hu = [bridge.tile([128, HL * PW], BF16, name=f"hu{ct}", tag=f"hu{ct}")
          for ct in range(2)]

    pa = ctx.enter_context(tc.tile_pool(name="pa", bufs=1))
    pb = ctx.enter_context(tc.tile_pool(name="pb", bufs=1))
    # one PSUM pool for everything: pp(4) + st(2) + p2(2) = 8 banks
    pap = ctx.enter_context(tc.tile_pool(name="pap", bufs=1, space="PSUM"))

    def pp_tile(n=512):
        return pap.tile([128, 512], F32, name="ppt", tag="pp", bufs=6)[:, :n]

    def st_tile(p=1, n=512):
        return pap.tile([128, 512], F32, name="stt", tag="pp",
                        bufs=6)[:p, :n]

    # ---------------- phase A1: source pooling ----------------
    sp = [pa.tile([128, NS], F32, name=f"sp{ct}", tag=f"sp{ct}")
          for ct in range(2)]
    for hc in range(4):  # 16 source rows -> 4 pooled rows per iteration
        for ct in range(2):
            sch = pa.tile([128, 16 * WF], F32, name="sch", tag="sch", bufs=2)
            sy.dma_start(out=sch.rearrange("p (h w) -> p h w", h=16),
                         in_=src.ap()[128 * ct:128 * (ct + 1),
                                      16 * hc:16 * (hc + 1), :])
            dst = sp[ct][:, 128 * hc:128 * (hc + 1)].rearrange(
                "p (h w) -> p h w", h=4)
            v.tensor_reduce(dst,
                            sch.rearrange("p (ph dh pw dw) -> p ph pw dh dw",
                                          ph=4, dh=4, dw=4), AX.XY, ALU.max)

    # ---------------- LN1 helper (channel-dim layernorm, token range) ----
    def ln1(toks, o, n, outs):
        ps_sum = st_tile(1, n)
        ps_ssq = st_tile(1, n)
        sq = pa.tile([128, 512], F32R, name="lsq", tag="lsq", bufs=1)[:, :n]
        for ct in range(2):
            tk = toks[ct][:, o:o + n]
            s.square(sq, tk)
            t.matmul(ps_ssq, r32(ones_col), r32(sq),
                     start=(ct == 0), stop=(ct == 1))
            t.matmul(ps_sum, ones_colF, tk, start=(ct == 0), stop=(ct == 1))
        mu = pa.tile([1, 512], F32R, name="mu", tag="mu", bufs=2)[:, :n]
        mu2 = pa.tile([1, 512], F32, name="mu2", tag="mu2", bufs=2)[:, :n]
        var = pa.tile([1, 512], F32, name="var", tag="var", bufs=2)[:, :n]
        std = pa.tile([1, 512], F32, name="std", tag="std", bufs=2)[:, :n]
        rs = pa.tile([1, 512], F32R, name="rs", tag="rs", bufs=2)[:, :n]
        v.tensor_scalar(mu, ps_sum, 1.0 / C, None, ALU.mult)
        v.tensor_tensor(mu2, f32(mu), f32(mu), ALU.mult)
        v.scalar_tensor_tensor(var, ps_ssq, 1.0 / C, mu2, ALU.mult,
                               ALU.subtract)
        s.activation(std, var, ACTF.Sqrt, bias=epsc, scale=1.0)
        v.reciprocal(rs, std)
        mu_b = pp_tile(n)
        rs_b = pp_tile(n)
        t.matmul(mu_b, r32(ones_row), r32(mu), start=True, stop=True)
        t.matmul(rs_b, r32(ones_row), r32(rs), start=True, stop=True)
        for ct in range(2):
            t1 = pa.tile([128, 512], F32, name="lt1", tag="lt1",
                         bufs=1)[:, :n]
            v.tensor_tensor(t1, toks[ct][:, o:o + n], mu_b, ALU.subtract)
            t2 = outs[ct][:, o:o + n]
            v.tensor_tensor(t2, t1, rs_b, ALU.mult)
            v.tensor_scalar(t2, t2, g1[ct], b1[ct], ALU.mult, ALU.add)

    s_ln = [pa.tile([128, NS], F32R, name=f"sln{ct}", tag=f"sln{ct}")
            for ct in range(2)]
    # ------- phase A3 state (x side, two row-halves) ----------------------
    xp = [pa.tile([128, NQ], F32, name=f"xp{ct}", tag=f"xp{ct}")
          for ct in range(2)]
    p_ln = [pa.tile([128, NQ], F32R, name=f"pln{ct}", tag=f"pln{ct}")
            for ct in range(2)]
    qp = [pa.tile([128, NQ], F32R, name=f"qp{ct}", tag=f"qp{ct}")
          for ct in range(2)]
    zz = pa.tile([8, NQ], F32R, name="zz", tag="zz")
    mz = [pa.tile([128, NQ], F32R, name=f"mz{ct}", tag=f"mz{ct}")
          for ct in range(2)]
    mp = [pa.tile([128, NQ], BF16, name=f"mp{ct}", tag=f"mp{ct}")
          for ct in range(2)]
    dh = [pa.tile([128, (PH - 1) * PW], BF16, name=f"dh{ct}", tag=f"dh{ct}")
          for ct in range(2)]

    def a_pool(hs):
        pr0, pn, o, n, k0 = HALVES[hs]
        subs = (4, 4, 2) if pn == 10 else (4, 4)
        p0 = pr0
        for spn in subs:  # DMA+pool chunks of spn pooled rows
            r0 = 4 * p0
            for ct in range(2):
                xch = pa.tile([128, 16 * WP], F32, name="xch", tag="xch",
                              bufs=2)[:, :4 * spn * WP]
                sy.dma_start(out=xch.rearrange("p (h w) -> p h w", h=4 * spn),
                             in_=xs.ap()[128 * ct:128 * (ct + 1),
                                         r0:r0 + 4 * spn, :])
                dst = xp[ct][:, p0 * PW:(p0 + spn) * PW].rearrange(
                    "p (h w) -> p h w", h=spn)
                v.tensor_reduce(dst,
                                xch.rearrange(
                                    "p (ph dh pw dw) -> p ph pw dh dw",
                                    ph=spn, dh=4, dw=4), AX.XY, ALU.max)
            p0 += spn

    a_pool(0)
    a_pool(1)
    ln1(sp, 0, NS, s_ln)

    # ---------------- phase A2: K/V token-major projections + KV ----------
    kp_tm, v_tm = [], []
    for st_ in range(4):  # 128-token tiles of the 512 source tokens
        psk = pp_tile(256)
        psv = pp_tile(256)
        for kt in range(2):
            lt = s_ln[kt][:, 128 * st_:128 * (st_ + 1)]
            t.matmul(psk, r32(lt), r32(wks[:, 256 * kt:256 * (kt + 1)]),
                     start=(kt == 0), stop=(kt == 1))
            t.matmul(psv, r32(lt), r32(wvs[:, 256 * kt:256 * (kt + 1)]),
                     start=(kt == 0), stop=(kt == 1))
        kp = pa.tile([128, 256], F32, name=f"kp{st_}", tag=f"kp{st_}")
        vv = pa.tile([128, 256], F32, name=f"vv{st_}", tag=f"vv{st_}")
        mn = pa.tile([128, 512], F32, name="mn", tag="eluT", bufs=4)[:, :256]
        s.activation(mn, psk, ACTF.Relu, scale=-1.0)  # relu(-x) = -min(x,0)
        ee = pa.tile([128, 512], F32, name="ee", tag="eluT", bufs=4)[:, :256]
        s.activation(ee, mn, ACTF.Exp, scale=-1.0)    # exp(min(x,0))
        v.scalar_tensor_tensor(kp, psk, 0.0, ee, ALU.max, ALU.add)
        s.copy(vv, psv)
        kp_tm.append(kp)
        v_tm.append(vv)

    # Ksum (channel-major [128,1] per ctile) and per-head KV in one bank
    kvks = pap.tile([128, 66], F32, name="kvks", tag="pp", bufs=6)
    ps_kv = [kvks[:, 0:32], kvks[:, 33:65]]
    ps_ks = [kvks[:, 32:33], kvks[:, 65:66]]
    for ct in range(2):
        for st_ in range(4):
            t.matmul(ps_ks[ct], kp_tm[st_][:, 128 * ct:128 * (ct + 1)],
                     ones_colF, start=(st_ == 0), stop=(st_ == 3))
    for ct in range(2):
        for hl in range(4):
            h = 4 * ct + hl
            for st_ in range(4):
                t.matmul(ps_kv[ct][32 * hl:32 * (hl + 1), :],
                         kp_tm[st_][:, 32 * h:32 * (h + 1)],
                         v_tm[st_][:, 32 * h:32 * (h + 1)],
                         start=(st_ == 0), stop=(st_ == 3),
                         tile_position=(0, 32 * hl))
    arb = [pa.tile([128, 33], F32, name=f"arb{ct}", tag=f"arb{ct}")
           for ct in range(2)]
    for ct in range(2):
        s.copy(arb[ct][:, 0:32], ps_kv[ct])
        s.copy(arb[ct][:, 32:33], ps_ks[ct])
        g.dma_start(out=cc_in[128 * ct:128 * (ct + 1), :], in_=arb[ct])

    if use_collective:
        g.collective_compute(
            "AllReduce", ALU.add,
            replica_groups=[[0, 1], [2, 3], [4, 5], [6, 7]],
            ins=[cc_in.opt()], outs=[cc_out.opt()])
    else:  # single-core cost-model build: plain copy stands in for AllReduce
        g.dma_start(out=cc_out.opt(), in_=cc_in.opt())

    arbr = [pa.tile([128, 33], F32, name=f"arbr{ct}", tag=f"arbr{ct}")
            for ct in range(2)]
    arbq = [pa.tile([128, 32], F32, name=f"arbq{ct}", tag=f"arbq{ct}")
            for ct in range(2)]
    for ct in range(2):
        g.dma_start(out=arbr[ct], in_=cc_out[128 * ct:128 * (ct + 1), :])
        s.copy(arbq[ct], arbr[ct][:, 0:32])
        for hl in range(4):
            s.copy(ksb[32 * hl:32 * (hl + 1),
                       12 * ct + hl:12 * ct + hl + 1],
                   arbr[ct][32 * hl:32 * (hl + 1), 32:33])

    def a_ln1(hs):
        pr0, pn, o, n, k0 = HALVES[hs]
        ln1(xp, o, n, p_ln)

    def a_q(hs):
        pr0, pn, o, n, k0 = HALVES[hs]
        for ct in range(2):
            psq = pp_tile(n)
            for kt in range(2):
                t.matmul(psq,
                         r32(wqs[:, 256 * kt + 128 * ct:
                                 256 * kt + 128 * ct + 128]),
                         r32(p_ln[kt][:, o:o + n]),
                         start=(kt == 0), stop=(kt == 1))
            mn = pa.tile([128, 512], F32, name="qmn", tag="eluT",
                         bufs=4)[:, :n]
            s.activation(mn, psq, ACTF.Relu, scale=-1.0)
            ee = pa.tile([128, 512], F32, name="qee", tag="eluT",
                         bufs=4)[:, :n]
            s.activation(ee, mn, ACTF.Exp, scale=-1.0)
            v.scalar_tensor_tensor(qp[ct][:, o:o + n], psq, 0.0, ee,
                                   ALU.max, ALU.add)

    def a_z(hs):
        pr0, pn, o, n, k0 = HALVES[hs]
        ps_z = st_tile(8, n)
        for ct in range(2):
            t.matmul(ps_z, r32(ksb[:, 8 * ct:8 * (ct + 1)]),
                     r32(qp[ct][:, o:o + n]), start=(ct == 0),
                     stop=(ct == 1))
        v.reciprocal(zz[:, o:o + n], ps_z)

    def a_zbmsg(hs):
        pr0, pn, o, n, k0 = HALVES[hs]
        for ct in range(2):
            ps_zb = pp_tile(n)
            t.matmul(ps_zb, r32(hb[:, 128 * ct:128 * (ct + 1)]),
                     r32(zz[:, o:o + n]), start=True, stop=True)
            zbs = pa.tile([128, 512], F32, name="zbs", tag="zbs",
                          bufs=2)[:, :n]
            s.copy(zbs, ps_zb)
            ps_m = pp_tile(n)
            for hl in range(4):
                t.matmul(ps_m[32 * hl:32 * (hl + 1), :],
                         arbq[ct][32 * hl:32 * (hl + 1), :],
                         f32(qp[ct][32 * hl:32 * (hl + 1), o:o + n]),
                         start=True, stop=True,
                         tile_position=(32 * hl, 32 * hl))
            v.tensor_tensor(mz[ct][:, o:o + n], ps_m, zbs, ALU.mult)

    def a_merge_hu(hs):
        pr0, pn, o, n, k0 = HALVES[hs]
        d0, dn = (0, 9) if hs == 0 else (9, 8)
        for ct in range(2):
            ps_g = pp_tile(n)
            for kt in range(2):
                t.matmul(ps_g,
                         r32(wms[:, 256 * kt + 128 * ct:
                                 256 * kt + 128 * ct + 128]),
                         r32(mz[kt][:, o:o + n]),
                         start=(kt == 0), stop=(kt == 1))
            s.copy(mp[ct][:, o:o + n], ps_g)
            m3 = mp[ct].rearrange("p (h w) -> p h w", h=PH)
            d3 = dh[ct].rearrange("p (h w) -> p h w", h=PH - 1)
            v.tensor_tensor(d3[:, d0:d0 + dn, :],
                            m3[:, d0 + 1:d0 + dn + 1, :],
                            m3[:, d0:d0 + dn, :], ALU.subtract)
            hu4 = hu[ct].rearrange("p (k r w) -> p k r w", r=4, w=PW)
            for r in range(4):
                sh, w = UPW[r]
                v.scalar_tensor_tensor(hu4[:, k0:k0 + 8, r, :],
                                       d3[:, k0 + sh:k0 + sh + 8, :], w,
                                       m3[:, k0 + sh:k0 + sh + 8, :],
                                       ALU.mult, ALU.add)

    # ------- phase B chunk (W-upsample pair + MLP + LN2 + residual) -------
    mf2_box = [None, None]

    def emit_chunk(kc):
        xc = []
        for ct in range(2):
            xct = pb.tile([128, CHUNK], F32R, name=f"xc{ct}", tag=f"xc{ct}",
                          bufs=2)
            g.dma_start(out=xct.rearrange("p (h w) -> p h w", h=4),
                        in_=xs.ap()[128 * ct:128 * (ct + 1),
                                    4 * kc + 4:4 * kc + 8, 4:4 + WF])
            xc.append(xct)
        xcr = xc

        if kc % 2 == 0:  # pair-batched W upsample of hu rows 4kc..4kc+7
            mf2 = []
            for ct in range(2):
                hu3 = hu[ct].rearrange("p (h w) -> p h w", h=HL)
                hus = hu3[:, 4 * kc:4 * kc + 8, :]
                dw = pb.tile([128, 8 * (PW - 1)], BF16, name="dw",
                             tag=f"dw{ct}", bufs=2)
                dw3 = dw.rearrange("p (h w) -> p h w", h=8)
                v.tensor_tensor(dw3, hus[:, :, 1:PW], hus[:, :, 0:PW - 1],
                                ALU.subtract)
                mft = pb.tile([128, 2 * CHUNK], F32R, name=f"mf{ct}",
                              tag=f"mf{ct}", bufs=2)
                mf4 = mft.rearrange("p (h j r) -> p h j r", j=32, r=4)
                for r in range(4):
                    sh, w = UPW[r]
                    v.scalar_tensor_tensor(mf4[:, :, :, r],
                                           dw3[:, :, sh:sh + 32], w,
                                           hus[:, :, sh:sh + 32],
                                           ALU.mult, ALU.add)
                mf2.append(mft)
            mf2_box[0], mf2_box[1] = mf2
        mf = [mft[:, 512 * (kc % 2):512 * (kc % 2) + 512]
              for mft in mf2_box]

        # MLP layer 1 + relu
        h1 = pb.tile([128, 2048], F32R, name="h1", tag="h1", bufs=2)
        cat = [xcr[0], xcr[1], mf[0], mf[1]]
        for m in range(4):
            ps1 = pp_tile()
            for kt in range(4):
                t.matmul(ps1,
                         r32(w1s[:, 512 * kt + 128 * m:
                                 512 * kt + 128 * m + 128]),
                         r32(cat[kt]), start=(kt == 0), stop=(kt == 3))
            s.activation(h1[:, 512 * m:512 * m + 512], ps1, ACTF.Relu)

        # MLP layer 2 (W2 pre-centered * g2), per-m psum tiles
        ps2 = []
        for m in range(2):
            p2 = pap.tile([128, CHUNK], F32, name="p2", tag="p2", bufs=2)
            for kt in range(4):
                t.matmul(p2,
                         r32(w2s[:, 256 * kt + 128 * m:
                                 256 * kt + 128 * m + 128]),
                         r32(h1[:, 512 * kt:512 * kt + 512]),
                         start=(kt == 0), stop=(kt == 3))
            ps2.append(p2)

        # LN2: rstd from channel sum of squares; PE row-broadcast of std
        ps_ss = st_tile(1)
        for ct in range(2):
            sq = pb.tile([128, CHUNK], F32R, name="sq", tag="sq", bufs=1)
            s.activation(sq, ps2[ct], ACTF.Square, scale=ig2[ct])
            t.matmul(ps_ss, r32(ones_col), r32(sq), start=(ct == 0),
                     stop=(ct == 1))
        std = pb.tile([1, CHUNK], F32R, name="std2", tag="std2", bufs=2)
        s.activation(std, ps_ss, ACTF.Sqrt, bias=epsc, scale=1.0 / C)
        ps_rb = pp_tile()
        t.matmul(ps_rb, r32(ones_row), r32(std), start=True, stop=True)
        rsb = pb.tile([128, CHUNK], F32, name="rsb", tag="rsb", bufs=2)
        v.reciprocal(rsb, ps_rb)

        for ct in range(2):
            tt = pb.tile([128, CHUNK], F32, name="tt", tag="tt", bufs=2)
            v.tensor_tensor(tt, ps2[ct], rsb, ALU.mult)
            # in-place: xc <- (tt + b2) + xc, then DMA out
            ocx = pb.tile([128, CHUNK], F32, name="ocx", tag=f"oc{ct}",
                          bufs=2)
            v.scalar_tensor_tensor(ocx, tt, b2[ct], f32(xc[ct]), ALU.add,
                                   ALU.add)
            sy.dma_start(out=out.ap()[128 * ct:128 * (ct + 1),
                                      4 * kc:4 * kc + 4, :],
                         in_=ocx.rearrange("p (h w) -> p h w", h=4))

    for stage in (a_ln1, a_q, a_z, a_zbmsg, a_merge_hu):
        for hs in range(2):
            stage(hs)
    for kc in range(NCH):
        emit_chunk(kc)


def build(use_collective=True):
    nc = bacc.Bacc("TRN2", target_bir_lowering=False, debug=False,
                   num_devices=8 if use_collective else 1)
    io = {
        "xs": nc.dram_tensor("xs", [C, HP, WP], F32, kind="ExternalInput"),
        "src": nc.dram_tensor("src", [C, HL, WF], F32, kind="ExternalInput"),
        "wq": nc.dram_tensor("wq", [C, C], F32, kind="ExternalInput"),
        "wk": nc.dram_tensor("wk", [C, C], F32, kind="ExternalInput"),
        "wv": nc.dram_tensor("wv", [C, C], F32, kind="ExternalInput"),
        "wm": nc.dram_tensor("wm", [C, C], F32, kind="ExternalInput"),
        "w1": nc.dram_tensor("w1", [2 * C, 2 * C], F32, kind="ExternalInput"),
        "w2": nc.dram_tensor("w2", [2 * C, C], F32, kind="ExternalInput"),
        "vecs": nc.dram_tensor("vecs", [C, 4], F32, kind="ExternalInput"),
        "hbm": nc.dram_tensor("hbm", [8, C], F32, kind="ExternalInput"),
        "out": nc.dram_tensor("out", [C, HL, WF], F32, kind="ExternalOutput"),
    }
    from contextlib import ExitStack
    with tile.TileContext(nc) as tc:
        with ExitStack() as ctx:
            ctx.enter_context(nc.allow_low_precision(
                reason="float32r rounding of matmul inputs is intentional"))
            _build_body(nc, tc, io, ctx, use_collective=use_collective)
    nc.compile()
    return nc


_NC = None


def _get_nc():
    global _NC
    if _NC is None:
        _NC = build()
    return _NC


def make_in_maps(inputs):
    x = np.ascontiguousarray(np.asarray(inputs["x"], np.float32))
    src = np.ascontiguousarray(np.asarray(inputs["source"], np.float32))
    g2 = np.asarray(inputs["norm2_g"], np.float32)
    b2 = np.asarray(inputs["norm2_b"], np.float32)
    w2 = np.asarray(inputs["mlp_w2"], np.float32)
    w2pp = np.ascontiguousarray((w2 - w2.mean(1, keepdims=True)) * g2[None, :])
    hbm = np.zeros((8, C), np.float32)
    for ct in range(2):
        for hl in range(4):
            hbm[4 * ct + hl, 128 * ct + 32 * hl:128 * ct + 32 * hl + 32] = 1.0
    ig2 = np.where(np.abs(g2) < 1e-12, 1.0, 1.0 / np.where(g2 == 0, 1, g2)
                   ).astype(np.float32)
    vecs = np.ascontiguousarray(np.stack(
        [np.asarray(inputs["norm1_g"], np.float32),
         np.asarray(inputs["norm1_b"], np.float32), b2, ig2], axis=1))
    shared = {
        "wq": np.ascontiguousarray(np.asarray(inputs["w_q"], np.float32)),
        "wk": np.ascontiguousarray(np.asarray(inputs["w_k"], np.float32)),
        "wv": np.ascontiguousarray(np.asarray(inputs["w_v"], np.float32)),
        "wm": np.ascontiguousarray(np.asarray(inputs["w_merge"], np.float32)),
        "w1": np.ascontiguousarray(np.asarray(inputs["mlp_w1"], np.float32)),
        "w2": w2pp, "vecs": vecs, "hbm": hbm,
    }
    in_maps = []
    for core in range(8):
        b, half = core // 2, core % 2
        r0 = 64 * half
        xb = x[b]
        rows = np.concatenate([
            xb[:, 0:4] if r0 == 0 else xb[:, r0 - 4:r0],
            xb[:, r0:r0 + 64],
            xb[:, 124:128] if r0 == 64 else xb[:, r0 + 64:r0 + 68],
        ], axis=1)
        xsh = np.ascontiguousarray(np.concatenate(
            [rows[:, :, 0:4], rows, rows[:, :, 124:128]], axis=2))
        ssh = np.ascontiguousarray(src[b][:, r0:r0 + 64])
        in_maps.append({"xs": xsh, "src": ssh, **shared})
    return in_maps


def run(inputs, **kwargs):
    nc = _get_nc()
    res = run_bass_kernel_spmd(nc, make_in_maps(inputs),
                               core_ids=list(range(8)), **kwargs)
    out = np.empty((4, C, 128, 128), np.float32)
    for core in range(8):
        b, half = core // 2, core % 2
        out[b, :, 64 * half:64 * half + 64] = res.results[core]["out"]
    return out, res


def kernel(**inputs):
    out, _ = run(inputs)
    return out

